# revision 18
# baseline (speedup 1.0000x reference)
"""SimCLR (NT-Xent) contrastive loss on 8 TRN2 NeuronCores.

reference semantics:
    xn = x / max(||x||, eps);  sim = xn @ xn.T;  sim[i,i] = -inf
    logits = sim / 0.5;  target(i) = i ^ 1
    loss = mean_i( logsumexp(logits[i,:]) - logits[i, target(i)] )

Distribution: data-parallel over rows of the similarity matrix. Each core
receives the full x^T (moving operand, bf16, pre-tiled [nt][p][k][n] so
every DMA is contiguous per partition) plus its own 512-column slice
(stationary operand), so the SPMD graph is identical on every core.
Norms are read off the diagonal of the raw Gram matrix (cheap extra phase
on [128,128] diagonal blocks), the 512-entry 1/norm vector is AllGathered
(tiny, on the HWDGE ring so it never queues behind bulk data), and the
epilogue fuses column-scaling (DVE) with exp + row-sum (one ACT op with
accum_out) per PSUM block. Host sums the 8 per-core partial losses.
"""

import numpy as np

try:
    import concourse.bass as bass
except ImportError:  # pragma: no cover
    import sys

    sys.path.insert(0, "/opt/trn_rl_repo")
    import concourse.bass as bass

import ml_dtypes
import concourse.mybir as mybir
from concourse import bacc, tile
from concourse.bass_utils import run_bass_kernel_spmd

B, D, NCORES = 4096, 1024, 8
RPC = B // NCORES  # rows per core (512)
KT = D // 128  # contraction chunks (8)
NT = B // 512  # moving-operand column tiles (8)
RC = RPC // 128  # 128-row chunks per core (4)
E2 = 7.38905609893065  # exp(sim_ii / T) with sim_ii == 1
F32 = mybir.dt.float32
BF16 = mybir.dt.bfloat16


def build(stage="full"):
    Alu = mybir.AluOpType
    Act = mybir.ActivationFunctionType
    nc = bacc.Bacc("TRN2", target_bir_lowering=False, num_devices=NCORES)

    xt = nc.dram_tensor("xt", [NT, 128, KT, 512], BF16, kind="ExternalInput")
    xo = nc.dram_tensor("xo", [128, KT, RPC], BF16, kind="ExternalInput")
    diagmask = nc.dram_tensor("diagmask", [128, 128], F32, kind="ExternalInput")
    pairmask = nc.dram_tensor("pairmask", [128, 128], F32, kind="ExternalInput")
    out = nc.dram_tensor("out", [1, 1], F32, kind="ExternalOutput")

    rn_local_d = nc.dram_tensor("rn_local_d", [RPC], F32, kind="Internal")
    rn_all_d = nc.dram_tensor(
        "rn_all_d", [B], F32, kind="Internal", addr_space="Shared"
    )

    with tile.TileContext(nc) as tc:
        with (
            tc.tile_pool(name="sb", bufs=1) as sb,
            tc.tile_pool(name="ps", bufs=7, space="PSUM") as psp,
            tc.tile_pool(name="aux", bufs=1, space="PSUM") as auxp,
        ):
            # ---- persistent SBUF tensors ----
            xo_sb = sb.tile([128, KT, RPC], BF16, tag="xo")
            strips = [
                sb.tile([128, KT, 512], BF16, tag=f"strip{i}", name=f"strip{i}")
                for i in range(NT)
            ]
            dmask = sb.tile([128, 128], F32, tag="dmask")
            pmask = sb.tile([128, 128], F32, tag="pmask")
            rn_bc = sb.tile([128, B], F32, tag="rnbc")
            ones128 = sb.tile([128, 1], F32, tag="ones128")
            n2 = sb.tile([128, RC], F32, tag="n2")
            n2r = sb.tile([128, RC], F32, tag="n2r")
            rn_loc = sb.tile([128, RC], F32, tag="rnloc")
            rn2_loc = sb.tile([128, RC], F32, tag="rn2loc")
            rn_swap = sb.tile([128, RC], F32, tag="rnswap")
            pairv = sb.tile([128, RC], F32, tag="pairv")
            zacc = sb.tile([128, RC * NT], F32, tag="zacc")

            # ---- input DMA: bulk on HWDGE (sync), contiguous per partition
            nc.sync.dma_start(xo_sb[:], xo[:])
            nc.sync.dma_start(dmask[:], diagmask[:])
            nc.sync.dma_start(pmask[:], pairmask[:])
            for ntb in range(NT):
                nc.sync.dma_start(strips[ntb][:], xt[ntb])
            nc.vector.memset(ones128[:], 1.0)
            neg_e2 = sb.tile([128, 1], F32, tag="nege2")
            nc.vector.memset(neg_e2[:], -E2)

            # ---- phase A: diagonal [128,128] Gram blocks -> norms + pairs
            for rc in range(RC):
                psA = psp.tile([128, 128], F32, tag="ps")
                own = xo_sb[:, :, rc * 128 : (rc + 1) * 128]
                for k in range(KT):
                    nc.tensor.matmul(
                        psA[:],
                        own[:, k, :],
                        own[:, k, :],
                        start=(k == 0),
                        stop=(k == KT - 1),
                    )
                jd = sb.tile([128, 128], F32, tag="junk128", bufs=2)
                nc.vector.tensor_mul(jd[:], psA[:], dmask[:])
                nc.vector.reduce_sum(
                    n2[:, rc : rc + 1], jd[:], axis=mybir.AxisListType.X
                )
                jp = sb.tile([128, 128], F32, tag="junk128", bufs=2)
                nc.vector.tensor_mul(jp[:], psA[:], pmask[:])
                nc.vector.reduce_sum(
                    pairv[:, rc : rc + 1], jp[:], axis=mybir.AxisListType.X
                )

            # rn = 1/sqrt(n2) (DVE reciprocal + ACT sqrt; ACT rsqrt is banned)
            nc.vector.reciprocal(n2r[:], n2[:])
            nc.scalar.activation(rn_loc[:], n2r[:], Act.Sqrt)
            nc.vector.tensor_scalar_mul(rn2_loc[:], rn_loc[:], 2.0)

            # partner-swapped rn via pair-permutation matmul
            psS = auxp.tile([128, RC], F32, tag="aux")
            nc.tensor.matmul(psS[:], pmask[:], rn_loc[:], start=True, stop=True)
            nc.vector.tensor_copy(rn_swap[:], psS[:])

            def finish(vec):
                ltot = sb.tile([128, 1], F32, tag="ltot", name="ltot")
                nc.vector.reduce_sum(ltot[:], vec, axis=mybir.AxisListType.X)
                psF = auxp.tile([1, 1], F32, tag="aux", name="psF")
                nc.tensor.matmul(psF[:], ones128[:], ltot[:], start=True, stop=True)
                osb = sb.tile([1, 1], F32, tag="osb", name="osb")
                nc.vector.tensor_copy(osb[:], psF[:])
                nc.sync.dma_start(out[:], osb[:])

            if stage == "A":
                finish(rn_loc[:])

            if stage != "A":
                # ---- AllGather of the 512-entry rn vector (HWDGE ring, so
                # these small DMAs are not FIFO-queued behind the strips)
                for rc in range(RC):
                    nc.sync.dma_start(rn_local_d[bass.ts(rc, 128)], rn_loc[:, rc])
                nc.gpsimd.collective_compute(
                    "AllGather",
                    Alu.bypass,
                    replica_groups=[list(range(NCORES))],
                    ins=[rn_local_d[:].opt()],
                    outs=[rn_all_d[:].opt()],
                )
                # broadcast-load rn to all 128 partitions via stride-0 DMA;
                # the only SWDGE transfer, so its queue is empty
                nc.gpsimd.dma_start(
                    rn_bc[:],
                    rn_all_d.rearrange("(a n) -> a n", a=1).to_broadcast([128, B]),
                )

            if stage == "B":
                finish(rn_bc[:, 0:8])

            if stage not in ("A", "B"):
                # ---- phase C: S matmuls + fused scale/exp/rowsum epilogue.
                # Strip 0's blocks are drained PSUM->SBUF immediately (no
                # dependency on the AllGather) and their epilogues deferred,
                # so the PE never stalls on PSUM while the gather completes.
                sdef = [
                    sb.tile([128, 512], F32, tag=f"sdef{i}", name=f"sdef{i}")
                    for i in range(RC)
                ]

                def mm_block(rcb, ntb):
                    ps = psp.tile([128, 512], F32, tag="ps", name="psC")
                    for k in range(KT):
                        nc.tensor.matmul(
                            ps[:],
                            xo_sb[:, k, rcb * 128 : (rcb + 1) * 128],
                            strips[ntb][:, k, :],
                            start=(k == 0),
                            stop=(k == KT - 1),
                        )
                    return ps

                def epilogue(src_ap, rcb, ntb):
                    scr = sb.tile([128, 512], F32, tag="scr", bufs=3, name="scr")
                    nc.vector.tensor_mul(
                        scr[:], src_ap, rn_bc[:, ntb * 512 : (ntb + 1) * 512]
                    )
                    jk = sb.tile([128, 512], F32, tag="junk512", bufs=2, name="jk")
                    col = rcb * NT + ntb
                    nc.scalar.activation(
                        jk[:],
                        scr[:],
                        Act.Exp,
                        scale=rn2_loc[:, rcb : rcb + 1],
                        accum_out=zacc[:, col : col + 1],
                    )

                for rcb in range(RC):
                    ps = mm_block(rcb, 0)
                    nc.vector.tensor_copy(sdef[rcb][:], ps[:])
                for ntb in range(1, NT):
                    for rcb in range(RC):
                        ps = mm_block(rcb, ntb)
                        epilogue(ps[:], rcb, ntb)
                for rcb in range(RC):
                    epilogue(sdef[rcb][:], rcb, 0)

            if stage == "C":
                finish(zacc[:, 0:RC])

            if stage == "full":
                # ---- phase D: per-row loss and final reduction ----
                zview = zacc[:].rearrange("p (a b) -> p a b", b=NT)
                zrow = sb.tile([128, RC], F32, tag="zrow")
                nc.vector.reduce_sum(zrow[:], zview, axis=mybir.AxisListType.X)
                lv = sb.tile([128, RC], F32, tag="lv")
                nc.scalar.activation(lv[:], zrow[:], Act.Ln, bias=neg_e2[:])
                t1 = sb.tile([128, RC], F32, tag="t1")
                nc.vector.tensor_mul(t1[:], pairv[:], rn_loc[:])
                t2 = sb.tile([128, RC], F32, tag="t2")
                nc.vector.tensor_mul(t2[:], t1[:], rn_swap[:])
                t3 = sb.tile([128, RC], F32, tag="t3")
                nc.vector.tensor_scalar_mul(t3[:], t2[:], 2.0)
                lossv = sb.tile([128, RC], F32, tag="lossv")
                nc.vector.tensor_sub(lossv[:], lv[:], t3[:])
                finish(lossv[:])

    nc.finalize()  # run bacc passes (register allocation etc.)
    return nc


_CACHE = {}


def get_built(stage="full"):
    if stage not in _CACHE:
        _CACHE[stage] = build(stage)
    return _CACHE[stage]


def make_in_maps(image: np.ndarray):
    image = np.asarray(image, dtype=np.float32)
    imT = np.ascontiguousarray(image.T).astype(ml_dtypes.bfloat16)  # [D, B]
    # [D, B] -> [KT, 128, NT, 512] -> tiled [NT, 128, KT, 512]
    xt_t = np.ascontiguousarray(
        imT.reshape(KT, 128, NT, 512).transpose(2, 1, 0, 3)
    )
    idx = np.arange(128)
    dmask = np.eye(128, dtype=np.float32)
    pmask = np.zeros((128, 128), dtype=np.float32)
    pmask[idx, idx ^ 1] = 1.0
    in_maps = []
    for c in range(NCORES):
        # own 512 columns in [p, k, n] layout (= xt tile c, n-major)
        xo_t = np.ascontiguousarray(xt_t[c])
        in_maps.append(
            {"xt": xt_t, "xo": xo_t, "diagmask": dmask, "pairmask": pmask}
        )
    return in_maps


def run(image: np.ndarray, stage="full", **spmd_kwargs):
    nc = get_built(stage)
    in_maps = make_in_maps(image)
    res = run_bass_kernel_spmd(
        nc, in_maps, core_ids=list(range(NCORES)), **spmd_kwargs
    )
    total = sum(float(r["out"][0, 0]) for r in res.results)
    return np.array(total / B, dtype=np.float32), res


def kernel(image: np.ndarray) -> np.ndarray:
    loss, _ = run(image)
    return loss
